# revision 32
# baseline (speedup 1.0000x reference)
"""Trainium2 Bass kernel for nn_JointLearningModel (coref-style joint model).

Sharding: the 384x384 pair grid is split by rows across 8 NeuronCores,
row i -> core i%8 (modulo sharding). Only the lower triangle j < i is
computed: with modulo sharding, local row k on any core has global index
8k+d (d<8), so a core-independent static column extent C_k =
roundup(8k+7, 32) covers every core's true extent and the per-core PE
work is identical (perfect balance). Columns beyond the true extent are
killed by the causal mask (-1e4) before the row softmax, which
underflows to exactly 0 in fp32.

Mention representations are gathered and transposed on the host (the
sharding hint treats all_mention_representations as replicated inputs);
params replicated; the scalar loss is summed on host across cores.

Schedule: weight DMAs are split across the SP and Activation HWDGE
queues; A.T is computed in two column halves so the main loop starts
~2.5us in; the mention-score MLP, character head, and softmax epilogue
are interleaved into the main loop as PE filler; the per-batch W3 score
reduction is deferred by one batch so PE never waits on the scalar
engine's relu output.
"""

import numpy as np
import ml_dtypes

import concourse.bass as bass
import concourse.mybir as mybir
import concourse.tile as tile
from concourse import bacc
from concourse.bass_utils import run_bass_kernel_spmd

F32 = mybir.dt.float32
BF16 = mybir.dt.bfloat16
FP8DT = mybir.dt.float8e4
PM = mybir.MatmulPerfMode
AF = mybir.ActivationFunctionType
OP = mybir.AluOpType

B, L, H, M = 8, 512, 768, 383
N = M + 1          # 384 rows/cols of the pair grid
NC_ = 8            # cores
R = N // NC_       # 48 rows per core
HC = H // 128      # 6 k-chunks of the hidden dim
NEG = -10000.0
FMAX = 512         # PSUM bank capacity in fp32 elements per partition
NH = 192           # A.T column half size

_CACHE = {}
LAST_RESULT = None

FP8 = False          # fp8e4 DoubleRow for the pair-MLP h2/w3 matmuls
WSCALE = 64.0       # fp8 weight pre-scale, compensated in the relu/copy


def _extent(k):
    """Static column extent for local row k (covers 8k+d for all d<8)."""
    return min(N, 32 * ((8 * k + 7 + 31) // 32))


def _batch_plan():
    """Pack rows into batches of segments with total F <= FMAX.

    Rows 0..11 (tiny extents) pack ascending; then each big row (desc
    from 47) pairs with the smallest-index unassigned rows that fit its
    remaining capacity. For this problem the packing is perfect: 20
    batches, 18 of them exactly F=512. Each batch is a list of segments
    (k0, G, C, off): G consecutive rows sharing extent C at offset off.
    """
    Cs = [_extent(k) for k in range(R)]

    def to_segments(rows):
        segs = []
        i = 0
        off = 0
        while i < len(rows):
            j = i
            while (j + 1 < len(rows) and rows[j + 1] == rows[j] + 1
                   and Cs[rows[j + 1]] == Cs[rows[i]]):
                j += 1
            g = j - i + 1
            segs.append((rows[i], g, Cs[rows[i]], off))
            off += g * Cs[rows[i]]
            i = j + 1
        return segs

    batches = []
    # early ascending fill over rows 0..11
    early = list(range(12))
    cur, cap = [], FMAX
    for k in early:
        if Cs[k] > cap:
            batches.append(to_segments(cur))
            cur, cap = [], FMAX
        cur.append(k)
        cap -= Cs[k]
    if cur:
        batches.append(to_segments(cur))
    # big rows descending, padded with smallest unassigned rows
    unassigned = list(range(12, R))
    while unassigned:
        big = unassigned.pop()          # largest index = largest extent
        cur, cap = [big], FMAX - Cs[big]
        while unassigned and Cs[unassigned[0]] <= cap:
            k = unassigned.pop(0)
            cur.append(k)
            cap -= Cs[k]
        batches.append(to_segments(sorted(cur)))
    assert sorted(k for b in batches for (k0, g, c, off) in b
                  for k in range(k0, k0 + g)) == list(range(R))
    return batches


BATCHES = _batch_plan()


def _batch_F(b):
    return sum(g * c for (k0, g, c, off) in b)


def _last_batch_with_row_ge(r):
    last = 0
    for i, b in enumerate(BATCHES):
        if any(k0 + g > r for (k0, g, c, off) in b):
            last = i
    return last


def _declare_inputs(nc):
    def din(name, shape, dt):
        return nc.dram_tensor(name, list(shape), dt, kind="ExternalInput")

    T = {}
    T["rT_in"] = din("rT_in", [128, HC, N], BF16)
    T["rTl_in"] = din("rTl_in", [128, HC, R], BF16)
    # waT split by output chunk for early-start loads
    for co in range(HC):
        T[f"waT{co}"] = din(f"waT{co}", [128, HC, 128], BF16)
    T["wbT"] = din("wbT", [128, HC, H], BF16)
    if FP8:
        T["w2p"] = din("w2p", [128, 3, 2, H // 2], FP8DT)
        T["w3p"] = din("w3p", [128, 2, 16], FP8DT)
        T["w3s"] = din("w3s", [128, 1], FP8DT)
    else:
        T["w2T"] = din("w2T", [128, HC, H // 2], BF16)
        T["w3c"] = din("w3c", [128, 3], BF16)
    T["b1c"] = din("b1c", [128, HC], F32)
    T["b2c"] = din("b2c", [128, 3], F32)
    T["wm1T"] = din("wm1T", [128, HC, H // 2], BF16)
    T["bm1c"] = din("bm1c", [128, 3], F32)
    T["wm2T"] = din("wm2T", [128, 3, H // 4], BF16)
    T["bm2c"] = din("bm2c", [128, 2], F32)
    T["wm3c"] = din("wm3c", [128, 2], BF16)
    T["wc1T"] = din("wc1T", [128, HC, H // 2], BF16)
    T["bc1c"] = din("bc1c", [128, 3], F32)
    T["wc2T"] = din("wc2T", [128, 3, 18], BF16)
    T["bc2r"] = din("bc2r", [1, 18], F32)
    T["maskb"] = din("maskb", [R, N], F32)
    T["multb"] = din("multb", [R, N], F32)
    T["wnll"] = din("wnll", [R, 1], F32)
    T["oneh"] = din("oneh", [R, 18], F32)
    T["wch"] = din("wch", [R, 1], F32)
    T["loss"] = nc.dram_tensor("loss", [1, 1], F32, kind="ExternalOutput")
    return T


def _emit_core(nc, tc, T, sfx, mainloop_reps=1):
    with tc.tile_pool(name=f"const{sfx}", bufs=1) as cp:
        def load(name, h, eng):
            t = cp.tile(list(h.shape), h.dtype, name=f"{name}{sfx}")
            eng.dma_start(out=t[:], in_=h.ap())
            return t

        # queue 1 (SP): what the first PE ops need, in order
        rT = load("rT", T["rT_in"], nc.sync)
        waTc = [load(f"waTc{co}", T[f"waT{co}"], nc.sync) for co in range(HC)]
        if FP8:
            w2p_sb = load("w2p_sb", T["w2p"], nc.sync)
            w3p_sb = load("w3p_sb", T["w3p"], nc.sync)
            w3s_sb = load("w3s_sb", T["w3s"], nc.sync)
        else:
            w2T_sb = load("w2T_sb", T["w2T"], nc.sync)
            w3c_sb = load("w3c_sb", T["w3c"], nc.sync)
        b1c_sb = load("b1c_sb", T["b1c"], nc.sync)
        b2c_sb = load("b2c_sb", T["b2c"], nc.sync)
        # queue 2 (Activation): everything else
        rTl = load("rTl", T["rTl_in"], nc.scalar)
        wbT_sb = load("wbT_sb", T["wbT"], nc.scalar)
        wm1T_sb = load("wm1T_sb", T["wm1T"], nc.scalar)
        bm1c_sb = load("bm1c_sb", T["bm1c"], nc.scalar)
        wm2T_sb = load("wm2T_sb", T["wm2T"], nc.scalar)
        bm2c_sb = load("bm2c_sb", T["bm2c"], nc.scalar)
        wm3c_sb = load("wm3c_sb", T["wm3c"], nc.scalar)
        wc1T_sb = load("wc1T_sb", T["wc1T"], nc.scalar)
        bc1c_sb = load("bc1c_sb", T["bc1c"], nc.scalar)
        wc2T_sb = load("wc2T_sb", T["wc2T"], nc.scalar)
        bc2r_sb = load("bc2r_sb", T["bc2r"], nc.scalar)
        maskb_sb = load("maskb_sb", T["maskb"], nc.scalar)
        multb_sb = load("multb_sb", T["multb"], nc.scalar)
        wnll_sb = load("wnll_sb", T["wnll"], nc.scalar)
        oneh_sb = load("oneh_sb", T["oneh"], nc.scalar)
        wch_sb = load("wch_sb", T["wch"], nc.scalar)

        one1 = cp.tile([1, R], F32)
        nc.vector.memset(one1[:], 1.0)

        at_sb = cp.tile([128, HC, N], BF16)    # A.T   (bf16)
        bb_sb = cp.tile([128, HC, R], F32)     # Bm.T + b1, local rows
        sblk = cp.tile([R, N], F32)            # assembled pair scores
        nc.vector.memset(sblk[:], 0.0)
        mskms = cp.tile([R, N], F32)           # mask + ms[j] broadcast
        ms1 = cp.tile([128, 3, N], BF16)
        ms2 = cp.tile([128, 2, N], BF16)
        ms_sb = cp.tile([1, N], F32)
        c1 = cp.tile([128, 3, R], BF16)
        clg = cp.tile([R, 18], F32)
        x = cp.tile([R, N], F32)
        pexp = cp.tile([R, N], F32)
        escr = cp.tile([R, N], F32)
        z = cp.tile([R, 1], F32)
        e = cp.tile([R, 1], F32)
        lz = cp.tile([R, 1], F32)
        le = cp.tile([R, 1], F32)
        tnll = cp.tile([R, 1], F32)
        cexp = cp.tile([R, 18], F32)
        cz = cp.tile([R, 1], F32)
        cscr = cp.tile([R, 18], F32)
        sl = cp.tile([R, 1], F32)
        lcz = cp.tile([R, 1], F32)
        cev = cp.tile([R, 1], F32)

        # ---------- preamble: A.T cols 0:NH, Bb ----------
        with tc.tile_pool(name=f"pre_ps{sfx}", bufs=1, space="PSUM") as pp:
            for co in range(HC):
                pa = pp.tile([128, NH], F32, tag="at", name=f"pa_{co}{sfx}",
                             bufs=2)
                for ci in range(HC):
                    nc.tensor.matmul(
                        out=pa[:],
                        lhsT=waTc[co][:, ci, :],
                        rhs=rT[:, ci, 0:NH],
                        start=(ci == 0),
                        stop=(ci == HC - 1),
                    )
                nc.scalar.copy(out=at_sb[:, co, 0:NH], in_=pa[:])
            for co in range(HC):
                pb = pp.tile([128, R], F32, tag="bb", name=f"pb_{co}{sfx}",
                             bufs=2)
                for ci in range(HC):
                    nc.tensor.matmul(
                        out=pb[:],
                        lhsT=wbT_sb[:, ci, co * 128 : (co + 1) * 128],
                        rhs=rTl[:, ci, :],
                        start=(ci == 0),
                        stop=(ci == HC - 1),
                    )
                nc.vector.tensor_scalar(
                    out=bb_sb[:, co, :],
                    in0=pb[:],
                    scalar1=b1c_sb[:, co : co + 1],
                    scalar2=None,
                    op0=OP.add,
                )

        # ---------- main loop + interleaved epilogue ----------
        with (
            tc.tile_pool(name=f"lp_sb{sfx}", bufs=1) as lsb,
            tc.tile_pool(name=f"lp_ps{sfx}", bufs=2, space="PSUM") as lps,
            tc.tile_pool(name=f"sr_ps{sfx}", bufs=1, space="PSUM") as sps,
            tc.tile_pool(name=f"ep_ps{sfx}", bufs=1, space="PSUM") as eps,
        ):
            def ep_tile(nm):
                return eps.tile([128, N], F32, tag="ep", name=f"{nm}{sfx}")

            def at_h2(co):
                pa = ep_tile(f"pa2_{co}")
                for ci in range(HC):
                    nc.tensor.matmul(
                        out=pa[:, 0 : N - NH],
                        lhsT=waTc[co][:, ci, :],
                        rhs=rT[:, ci, NH:N],
                        start=(ci == 0),
                        stop=(ci == HC - 1),
                    )
                nc.scalar.copy(out=at_sb[:, co, NH:N], in_=pa[:, 0 : N - NH])

            def ms1_co(co):
                pm = ep_tile(f"pm_{co}")
                for ci in range(HC):
                    nc.tensor.matmul(
                        out=pm[:],
                        lhsT=wm1T_sb[:, ci, co * 128 : (co + 1) * 128],
                        rhs=rT[:, ci, :],
                        start=(ci == 0),
                        stop=(ci == HC - 1),
                    )
                nc.scalar.activation(
                    out=ms1[:, co, :], in_=pm[:], func=AF.Relu,
                    bias=bm1c_sb[:, co : co + 1],
                )

            def ms2_co(co):
                sz = (128, 64)[co]
                pm2 = ep_tile(f"pm2_{co}")
                for ci in range(3):
                    nc.tensor.matmul(
                        out=pm2[:sz, :],
                        lhsT=wm2T_sb[:, ci, co * 128 : co * 128 + sz],
                        rhs=ms1[:, ci, :],
                        start=(ci == 0),
                        stop=(ci == 2),
                    )
                nc.scalar.activation(
                    out=ms2[:sz, co, :], in_=pm2[:sz, :], func=AF.Relu,
                    bias=bm2c_sb[:sz, co : co + 1],
                )

            def ms3_mskms():
                pms = ep_tile("pms")
                nc.tensor.matmul(
                    out=pms[0:1, :], lhsT=wm3c_sb[:, 0:1], rhs=ms2[:, 0, :],
                    start=True, stop=False,
                )
                nc.tensor.matmul(
                    out=pms[0:1, :], lhsT=wm3c_sb[:64, 1:2], rhs=ms2[:64, 1, :],
                    start=False, stop=True,
                )
                nc.vector.tensor_copy(out=ms_sb[:], in_=pms[0:1, :])
                pbc = ep_tile("pbc")
                nc.tensor.matmul(
                    out=pbc[0:R, :], lhsT=one1[:], rhs=ms_sb[:],
                    start=True, stop=True,
                )
                nc.vector.tensor_tensor(
                    out=mskms[:], in0=pbc[0:R, :], in1=maskb_sb[:], op=OP.add
                )

            def char1_co(co):
                pc = ep_tile(f"pc_{co}")
                for ci in range(HC):
                    nc.tensor.matmul(
                        out=pc[:, 0:R],
                        lhsT=wc1T_sb[:, ci, co * 128 : (co + 1) * 128],
                        rhs=rTl[:, ci, :],
                        start=(ci == 0),
                        stop=(ci == HC - 1),
                    )
                nc.scalar.activation(
                    out=c1[:, co, :], in_=pc[:, 0:R], func=AF.Relu,
                    bias=bc1c_sb[:, co : co + 1],
                )

            def char_lg():
                plg = ep_tile("plg")
                for co in range(3):
                    nc.tensor.matmul(
                        out=plg[0:R, 0:18], lhsT=c1[:, co, :],
                        rhs=wc2T_sb[:, co, :],
                        start=(co == 0), stop=False,
                    )
                nc.tensor.matmul(
                    out=plg[0:R, 0:18], lhsT=one1[:], rhs=bc2r_sb[:],
                    start=False, stop=True,
                )
                nc.vector.tensor_copy(out=clg[:], in_=plg[0:R, 0:18])

            def char_sm_a():
                # scores are O(1); no row-max subtraction needed in fp32
                nc.scalar.activation(
                    out=cexp[:], in_=clg[:], func=AF.Exp, accum_out=cz[:],
                )

            def char_sm_b():
                nc.vector.tensor_tensor(
                    out=cscr[:], in0=clg[:], in1=oneh_sb[:], op=OP.mult
                )
                nc.vector.tensor_reduce(
                    out=sl[:], in_=cscr[:], axis=mybir.AxisListType.X, op=OP.add
                )

            def nll_exp(r0, r1):
                s = slice(r0, r1)
                nc.vector.tensor_tensor(
                    out=x[s, :], in0=sblk[s, :], in1=mskms[s, :], op=OP.add
                )
                nc.scalar.activation(
                    out=pexp[s, :], in_=x[s, :], func=AF.Exp,
                    accum_out=z[s, :],
                )
                nc.vector.tensor_tensor(
                    out=escr[s, :], in0=pexp[s, :], in1=multb_sb[s, :],
                    op=OP.mult,
                )
                nc.vector.tensor_reduce(
                    out=e[s, :], in_=escr[s, :], axis=mybir.AxisListType.X,
                    op=OP.add,
                )

            def ln_block():
                # all Ln ops back-to-back: one activation-table switch
                nc.scalar.activation(out=lz[:], in_=z[:], func=AF.Ln)
                nc.scalar.activation(out=le[:], in_=e[:], func=AF.Ln)
                nc.scalar.activation(out=lcz[:], in_=cz[:], func=AF.Ln)
                nc.vector.tensor_tensor(
                    out=tnll[:], in0=lz[:], in1=le[:], op=OP.subtract
                )
                nc.vector.tensor_tensor(
                    out=cev[:], in0=lcz[:], in1=sl[:], op=OP.subtract
                )

            nll2_at = _last_batch_with_row_ge(32) + 1
            extras = {
                0: [lambda: at_h2(0), lambda: at_h2(1), lambda: at_h2(2)],
                1: [lambda: at_h2(3), lambda: at_h2(4), lambda: at_h2(5)],
                2: [lambda: ms1_co(0)],
                3: [lambda: ms1_co(1)],
                4: [lambda: ms1_co(2)],
                5: [lambda: ms2_co(0)],
                6: [lambda: ms2_co(1)],
                7: [ms3_mskms],
                8: [lambda: char1_co(0)],
                9: [lambda: char1_co(1)],
                10: [lambda: char1_co(2)],
                11: [char_lg],
                12: [char_sm_a],
                13: [char_sm_b],
                nll2_at: [lambda: nll_exp(32, R)],
            }

            pend = None  # (h2sa tile, F, segs, idx) awaiting W3 reduce

            def flush_pend():
                h2sa, F, segs, bi = pend
                sr = sps.tile([1, FMAX], F32, tag="srow", name=f"sr_{bi}{sfx}",
                              bufs=1)
                if FP8:
                    nc.tensor.matmul(
                        out=sr[:, :F], lhsT=w3p_sb[:, :, 0:1],
                        rhs=h2sa[:, 0:2, :F],
                        start=True, stop=False, perf_mode=PM.DoubleRow,
                    )
                    nc.tensor.matmul(
                        out=sr[:, :F], lhsT=w3s_sb[:, 0:1],
                        rhs=h2sa[:, 2, :F],
                        start=False, stop=True,
                    )
                else:
                    for hb in range(3):
                        nc.tensor.matmul(
                            out=sr[:, :F],
                            lhsT=w3c_sb[:, hb : hb + 1],
                            rhs=h2sa[:, hb, :F],
                            start=(hb == 0),
                            stop=(hb == 2),
                        )
                srow = lsb.tile([1, FMAX], F32, tag="srow_sb",
                                name=f"srow_{bi}{sfx}", bufs=2)
                nc.scalar.mul(out=srow[:, :F], in_=sr[:, :F],
                              mul=(1.0 / WSCALE) if FP8 else 1.0)
                for (k0, G, C, off) in segs:
                    nc.sync.dma_start(
                        out=sblk[k0 : k0 + G, 0:C],
                        in_=srow[0:1, off : off + G * C],
                    )

            H1DT = FP8DT if FP8 else BF16
            for pass_ in range(mainloop_reps):
                for bi, segs in enumerate(BATCHES):
                    F = _batch_F(segs)
                    h1 = lsb.tile([128, HC, FMAX], H1DT, tag="h1",
                                  name=f"h1_{pass_}_{bi}{sfx}", bufs=3)
                    for c in range(HC):
                        for (k0, G, C, off) in segs:
                            for g in range(G):
                                nc.vector.tensor_scalar(
                                    out=h1[:, c, off + g * C : off + (g + 1) * C],
                                    in0=at_sb[:, c, 0:C],
                                    scalar1=bb_sb[:, c, k0 + g : k0 + g + 1],
                                    scalar2=0.0,
                                    op0=OP.add,
                                    op1=OP.max,
                                )
                    h2sa = lsb.tile([128, 3, FMAX], H1DT, tag="h2sa",
                                    name=f"hs_{pass_}_{bi}{sfx}", bufs=2)
                    for hb in range(3):
                        ph = lps.tile([128, FMAX], F32, tag=f"h2_{hb}",
                                      name=f"ph_{pass_}_{bi}_{hb}{sfx}")
                        if FP8:
                            for kp in range(3):
                                nc.tensor.matmul(
                                    out=ph[:, :F],
                                    lhsT=w2p_sb[:, kp, :,
                                                hb * 128 : (hb + 1) * 128],
                                    rhs=h1[:, 2 * kp : 2 * kp + 2, 0:F],
                                    start=(kp == 0),
                                    stop=(kp == 2),
                                    perf_mode=PM.DoubleRow,
                                )
                        else:
                            for c in range(HC):
                                nc.tensor.matmul(
                                    out=ph[:, :F],
                                    lhsT=w2T_sb[:, c, hb * 128 : (hb + 1) * 128],
                                    rhs=h1[:, c, 0:F],
                                    start=(c == 0),
                                    stop=(c == HC - 1),
                                )
                        nc.scalar.activation(
                            out=h2sa[:, hb, :F], in_=ph[:, :F], func=AF.Relu,
                            bias=b2c_sb[:, hb : hb + 1],
                            scale=(1.0 / WSCALE) if FP8 else 1.0,
                        )
                    if pend is not None:
                        flush_pend()
                    pend = (h2sa, F, segs, f"{pass_}_{bi}")
                    if pass_ == 0:
                        for fn in extras.get(bi, []):
                            fn()
            flush_pend()

            # ---------- tail ----------
            nll_exp(0, 32)
            ln_block()
            pl = ep_tile("pl")
            nc.tensor.matmul(
                out=pl[0:1, 0:1], lhsT=tnll[:, 0:1], rhs=wnll_sb[:],
                start=True, stop=False,
            )
            nc.tensor.matmul(
                out=pl[0:1, 0:1], lhsT=cev[:, 0:1], rhs=wch_sb[:],
                start=False, stop=True,
            )
            lout = cp.tile([1, 1], F32)
            nc.vector.tensor_copy(out=lout[:], in_=pl[0:1, 0:1])
            nc.sync.dma_start(out=T["loss"].ap(), in_=lout[:])


def _build_program(repeat=1, mainloop_reps=1):
    nc = bacc.Bacc(
        "TRN2", target_bir_lowering=False, debug=False, enable_asserts=False
    )
    T = _declare_inputs(nc)
    with tile.TileContext(nc) as tc:
        for rep in range(repeat):
            _emit_core(nc, tc, T, f"_r{rep}" if repeat > 1 else "",
                       mainloop_reps=mainloop_reps)
    nc.compile()
    return nc


def _chunk_cols(w):
    """[K, O] -> [128, K//128, O]  (partition-chunked contraction dim)."""
    k, o = w.shape
    return np.ascontiguousarray(w.reshape(k // 128, 128, o).transpose(1, 0, 2))


def _chunk_vec(v, ncol):
    """[C] -> [128, ncol] column-chunks (zero padded)."""
    out = np.zeros((128, ncol), np.float32)
    for c in range(ncol):
        seg = v[c * 128 : (c + 1) * 128]
        out[: len(seg), c] = seg
    return out


def _prep_in_maps(inputs):
    bf = ml_dtypes.bfloat16

    seq = np.asarray(inputs["sequence_output"], np.float32)
    spk = np.asarray(inputs["speaker_emb"], np.float32)
    dummy = np.asarray(inputs["dummy_emb"], np.float32)

    seg = np.asarray(inputs["mentions_seg"]).astype(np.int64)
    mstart = np.asarray(inputs["mention_start"]).astype(np.int64)
    mend = np.asarray(inputs["mention_end"]).astype(np.int64)
    sid = np.asarray(inputs["speaker_ids"]).astype(np.int64)[seg, mstart]
    mention_reps = seq[seg, mstart] + seq[seg, mend] + spk[sid]  # [M, H] f32
    all_reps = np.concatenate([dummy, mention_reps], axis=0)     # [N, H]
    # rT[p, c, m] = all_reps[m, c*128+p]
    rT_np = np.ascontiguousarray(
        all_reps.reshape(N, HC, 128).transpose(2, 1, 0)
    ).astype(bf)                                                 # [128, HC, N]

    W_pair1 = np.asarray(inputs["W_pair1"], np.float32)
    waT = _chunk_cols(np.ascontiguousarray(W_pair1[:, :H].T)).astype(bf)
    wbT = _chunk_cols(np.ascontiguousarray(W_pair1[:, H:].T)).astype(bf)
    w2T = _chunk_cols(
        np.ascontiguousarray(np.asarray(inputs["W_pair2"], np.float32).T)
    )                                                            # [128, 6, 384] f32
    w3c = _chunk_vec(np.asarray(inputs["W_pair3"], np.float32)[0], 3)
    if FP8:
        f8 = mybir.dt.np(FP8DT)
        w2p = np.ascontiguousarray(
            (w2T * WSCALE).reshape(128, 3, 2, H // 2)
        ).astype(f8)                                             # paired k-chunks
        w3p = np.zeros((128, 2, 16), np.float32)
        w3p[:, 0, 0] = w3c[:, 0] * WSCALE
        w3p[:, 1, 0] = w3c[:, 1] * WSCALE
        w3p = w3p.astype(f8)
        w3s = (w3c[:, 2:3] * WSCALE).astype(f8)
    w2T = w2T.astype(bf)
    w3c = w3c.astype(bf)
    b1c = _chunk_vec(np.asarray(inputs["b_pair1"], np.float32), HC)
    b2c = _chunk_vec(np.asarray(inputs["b_pair2"], np.float32), 3)
    wm1T = _chunk_cols(
        np.ascontiguousarray(np.asarray(inputs["W_m1"], np.float32).T)
    ).astype(bf)
    bm1c = _chunk_vec(np.asarray(inputs["b_m1"], np.float32), 3)
    wm2T = _chunk_cols(
        np.ascontiguousarray(np.asarray(inputs["W_m2"], np.float32).T)
    ).astype(bf)
    bm2c = _chunk_vec(np.asarray(inputs["b_m2"], np.float32), 2)
    wm3c = _chunk_vec(np.asarray(inputs["W_m3"], np.float32)[0], 2).astype(bf)
    wc1T = _chunk_cols(
        np.ascontiguousarray(np.asarray(inputs["W_c1"], np.float32).T)
    ).astype(bf)
    bc1c = _chunk_vec(np.asarray(inputs["b_c1"], np.float32), 3)
    wc2T = _chunk_cols(
        np.ascontiguousarray(np.asarray(inputs["W_c2"], np.float32).T)
    ).astype(bf)
    bc2r = np.asarray(inputs["b_c2"], np.float32).reshape(1, 18)

    link_first = np.asarray(inputs["link_first"]).astype(np.int64)
    link_second = np.asarray(inputs["link_second"]).astype(np.int64)
    label = np.asarray(inputs["character_label"]).astype(np.int64)

    mult = np.zeros((N, N), np.float32)
    np.add.at(mult, (link_second, link_first), 1.0)
    has_link = mult.sum(axis=1) > 0
    wnll_full = ((np.arange(N) >= 1) & has_link).astype(np.float32)
    mult[~has_link, 0] = 1.0  # keep log(E) finite; weight is 0 there

    mask_full = np.where(
        np.arange(N)[None, :] >= np.arange(N)[:, None], np.float32(NEG), 0.0
    ).astype(np.float32)

    oneh_full = np.zeros((N, 18), np.float32)
    wch_full = np.zeros(N, np.float32)
    oneh_full[np.arange(1, N), label] = 1.0
    wch_full[1:] = 1.0

    shared = dict(
        rT_in=rT_np,
        wbT=wbT, b1c=b1c, b2c=b2c,
        wm1T=wm1T, bm1c=bm1c, wm2T=wm2T, bm2c=bm2c, wm3c=wm3c,
        wc1T=wc1T, bc1c=bc1c, wc2T=wc2T, bc2r=bc2r,
    )
    if FP8:
        shared.update(w2p=w2p, w3p=w3p, w3s=w3s)
    else:
        shared.update(w2T=w2T, w3c=w3c)
    for co in range(HC):
        shared[f"waT{co}"] = np.ascontiguousarray(
            waT[:, :, co * 128 : (co + 1) * 128]
        )
    in_maps = []
    for d in range(NC_):
        rows = np.arange(R) * NC_ + d      # modulo sharding: row 8k+d
        m = dict(shared)
        m["rTl_in"] = np.ascontiguousarray(rT_np[:, :, rows])
        m["maskb"] = np.ascontiguousarray(mask_full[rows])
        if d == 0:
            # Global row 0 is fully masked; without row-max subtraction its
            # Z would be exactly 0 (ln -> -inf). Unmask its [0,0] entry:
            # mult[0,0]=1 and wnll[0]=0 make the row contribute exactly 0.
            m["maskb"][0, 0] = 0.0
        m["multb"] = np.ascontiguousarray(mult[rows])
        m["wnll"] = np.ascontiguousarray(wnll_full[rows]).reshape(R, 1)
        m["oneh"] = np.ascontiguousarray(oneh_full[rows])
        m["wch"] = np.ascontiguousarray(wch_full[rows]).reshape(R, 1)
        in_maps.append(m)
    return in_maps


def kernel(**inputs):
    global LAST_RESULT
    in_maps = _prep_in_maps(inputs)

    if "nc" not in _CACHE:
        _CACHE["nc"] = _build_program()
    nc = _CACHE["nc"]

    res = run_bass_kernel_spmd(nc, in_maps, core_ids=list(range(NC_)))
    LAST_RESULT = res
    total = np.float32(0.0)
    for d in range(NC_):
        total += np.float32(res.results[d]["loss"][0, 0])
    return np.asarray(total, dtype=np.float32)


if __name__ == "__main__":
    import reference

    inputs = {k: np.asarray(v) for k, v in reference.setup_inputs().items()}
    out = kernel(**inputs)
    print("kernel out:", out)


# revision 33
# speedup vs baseline: 1.0686x; 1.0686x over previous
"""Trainium2 Bass kernel for nn_JointLearningModel (coref-style joint model).

Sharding: the 384x384 pair grid is split by rows across 8 NeuronCores,
row i -> core i%8 (modulo sharding). Only the lower triangle j < i is
computed: with modulo sharding, local row k on any core has global index
8k+d (d<8), so a core-independent static column extent C_k =
roundup(8k+7, 32) covers every core's true extent and the per-core PE
work is identical (perfect balance). Columns beyond the true extent are
killed by the causal mask (-1e4) before the row softmax, which
underflows to exactly 0 in fp32.

Mention representations are gathered and transposed on the host (the
sharding hint treats all_mention_representations as replicated inputs);
params replicated; the scalar loss is summed on host across cores.

Schedule: weight DMAs are split across the SP and Activation HWDGE
queues; A.T is computed in two column halves so the main loop starts
~2.5us in; the mention-score MLP, character head, and softmax epilogue
are interleaved into the main loop as PE filler; the per-batch W3 score
reduction is deferred by one batch so PE never waits on the scalar
engine's relu output.
"""

import numpy as np
import ml_dtypes

import concourse.bass as bass
import concourse.mybir as mybir
import concourse.tile as tile
from concourse import bacc
from concourse.bass_utils import run_bass_kernel_spmd

F32 = mybir.dt.float32
BF16 = mybir.dt.bfloat16
FP8DT = mybir.dt.float8e4
PM = mybir.MatmulPerfMode
AF = mybir.ActivationFunctionType
OP = mybir.AluOpType

B, L, H, M = 8, 512, 768, 383
N = M + 1          # 384 rows/cols of the pair grid
NC_ = 8            # cores
R = N // NC_       # 48 rows per core
HC = H // 128      # 6 k-chunks of the hidden dim
NEG = -10000.0
FMAX = 512         # PSUM bank capacity in fp32 elements per partition
NH = 192           # A.T column half size

_CACHE = {}
LAST_RESULT = None

FP8 = True          # fp8e4 DoubleRow for the pair-MLP h2/w3 matmuls
WSCALE = 64.0       # fp8 weight pre-scale, compensated in the relu/copy


def _extent(k):
    """Static column extent for local row k (covers 8k+d for all d<8)."""
    return min(N, 32 * ((8 * k + 7 + 31) // 32))


def _batch_plan():
    """Pack rows into batches of segments with total F <= FMAX.

    Rows 0..11 (tiny extents) pack ascending; then each big row (desc
    from 47) pairs with the smallest-index unassigned rows that fit its
    remaining capacity. For this problem the packing is perfect: 20
    batches, 18 of them exactly F=512. Each batch is a list of segments
    (k0, G, C, off): G consecutive rows sharing extent C at offset off.
    """
    Cs = [_extent(k) for k in range(R)]

    def to_segments(rows):
        segs = []
        i = 0
        off = 0
        while i < len(rows):
            j = i
            while (j + 1 < len(rows) and rows[j + 1] == rows[j] + 1
                   and Cs[rows[j + 1]] == Cs[rows[i]]):
                j += 1
            g = j - i + 1
            segs.append((rows[i], g, Cs[rows[i]], off))
            off += g * Cs[rows[i]]
            i = j + 1
        return segs

    batches = []
    # early ascending fill over rows 0..11
    early = list(range(12))
    cur, cap = [], FMAX
    for k in early:
        if Cs[k] > cap:
            batches.append(to_segments(cur))
            cur, cap = [], FMAX
        cur.append(k)
        cap -= Cs[k]
    if cur:
        batches.append(to_segments(cur))
    # big rows descending, padded with smallest unassigned rows
    unassigned = list(range(12, R))
    while unassigned:
        big = unassigned.pop()          # largest index = largest extent
        cur, cap = [big], FMAX - Cs[big]
        while unassigned and Cs[unassigned[0]] <= cap:
            k = unassigned.pop(0)
            cur.append(k)
            cap -= Cs[k]
        batches.append(to_segments(sorted(cur)))
    assert sorted(k for b in batches for (k0, g, c, off) in b
                  for k in range(k0, k0 + g)) == list(range(R))
    return batches


BATCHES = _batch_plan()


def _batch_F(b):
    return sum(g * c for (k0, g, c, off) in b)


def _last_batch_with_row_ge(r):
    last = 0
    for i, b in enumerate(BATCHES):
        if any(k0 + g > r for (k0, g, c, off) in b):
            last = i
    return last


def _declare_inputs(nc):
    def din(name, shape, dt):
        return nc.dram_tensor(name, list(shape), dt, kind="ExternalInput")

    T = {}
    T["rT_in"] = din("rT_in", [128, HC, N], BF16)
    T["rTl_in"] = din("rTl_in", [128, HC, R], BF16)
    # waT split by output chunk for early-start loads
    for co in range(HC):
        T[f"waT{co}"] = din(f"waT{co}", [128, HC, 128], BF16)
    T["wbT"] = din("wbT", [128, HC, H], BF16)
    if FP8:
        T["w2p"] = din("w2p", [128, 3, 2, H // 2], FP8DT)
        T["w3p"] = din("w3p", [128, 2, 16], FP8DT)
        T["w3s"] = din("w3s", [128, 1], FP8DT)
    else:
        T["w2T"] = din("w2T", [128, HC, H // 2], BF16)
        T["w3c"] = din("w3c", [128, 3], BF16)
    T["b1c"] = din("b1c", [128, HC], F32)
    T["b2c"] = din("b2c", [128, 3], F32)
    T["wm1T"] = din("wm1T", [128, HC, H // 2], BF16)
    T["bm1c"] = din("bm1c", [128, 3], F32)
    T["wm2T"] = din("wm2T", [128, 3, H // 4], BF16)
    T["bm2c"] = din("bm2c", [128, 2], F32)
    T["wm3c"] = din("wm3c", [128, 2], BF16)
    T["wc1T"] = din("wc1T", [128, HC, H // 2], BF16)
    T["bc1c"] = din("bc1c", [128, 3], F32)
    T["wc2T"] = din("wc2T", [128, 3, 18], BF16)
    T["bc2r"] = din("bc2r", [1, 18], F32)
    T["maskb"] = din("maskb", [R, N], F32)
    T["multb"] = din("multb", [R, N], F32)
    T["wnll"] = din("wnll", [R, 1], F32)
    T["oneh"] = din("oneh", [R, 18], F32)
    T["wch"] = din("wch", [R, 1], F32)
    T["loss"] = nc.dram_tensor("loss", [1, 1], F32, kind="ExternalOutput")
    return T


def _emit_core(nc, tc, T, sfx, mainloop_reps=1):
    with tc.tile_pool(name=f"const{sfx}", bufs=1) as cp:
        def load(name, h, eng):
            t = cp.tile(list(h.shape), h.dtype, name=f"{name}{sfx}")
            eng.dma_start(out=t[:], in_=h.ap())
            return t

        # queue 1 (SP): what the first PE ops need, in order
        rT = load("rT", T["rT_in"], nc.sync)
        waTc = [load(f"waTc{co}", T[f"waT{co}"], nc.sync) for co in range(HC)]
        if FP8:
            w2p_sb = load("w2p_sb", T["w2p"], nc.sync)
            w3p_sb = load("w3p_sb", T["w3p"], nc.sync)
            w3s_sb = load("w3s_sb", T["w3s"], nc.sync)
        else:
            w2T_sb = load("w2T_sb", T["w2T"], nc.sync)
            w3c_sb = load("w3c_sb", T["w3c"], nc.sync)
        b1c_sb = load("b1c_sb", T["b1c"], nc.sync)
        b2c_sb = load("b2c_sb", T["b2c"], nc.sync)
        # queue 2 (Activation): everything else
        rTl = load("rTl", T["rTl_in"], nc.scalar)
        wbT_sb = load("wbT_sb", T["wbT"], nc.scalar)
        wm1T_sb = load("wm1T_sb", T["wm1T"], nc.scalar)
        bm1c_sb = load("bm1c_sb", T["bm1c"], nc.scalar)
        wm2T_sb = load("wm2T_sb", T["wm2T"], nc.scalar)
        bm2c_sb = load("bm2c_sb", T["bm2c"], nc.scalar)
        wm3c_sb = load("wm3c_sb", T["wm3c"], nc.scalar)
        wc1T_sb = load("wc1T_sb", T["wc1T"], nc.scalar)
        bc1c_sb = load("bc1c_sb", T["bc1c"], nc.scalar)
        wc2T_sb = load("wc2T_sb", T["wc2T"], nc.scalar)
        bc2r_sb = load("bc2r_sb", T["bc2r"], nc.scalar)
        maskb_sb = load("maskb_sb", T["maskb"], nc.scalar)
        multb_sb = load("multb_sb", T["multb"], nc.scalar)
        wnll_sb = load("wnll_sb", T["wnll"], nc.scalar)
        oneh_sb = load("oneh_sb", T["oneh"], nc.scalar)
        wch_sb = load("wch_sb", T["wch"], nc.scalar)

        one1 = cp.tile([1, R], F32)
        nc.vector.memset(one1[:], 1.0)

        at_sb = cp.tile([128, HC, N], BF16)    # A.T   (bf16)
        bb_sb = cp.tile([128, HC, R], F32)     # Bm.T + b1, local rows
        sblk = cp.tile([R, N], F32)            # assembled pair scores
        nc.vector.memset(sblk[:], 0.0)
        mskms = cp.tile([R, N], F32)           # mask + ms[j] broadcast
        ms1 = cp.tile([128, 3, N], BF16)
        ms2 = cp.tile([128, 2, N], BF16)
        ms_sb = cp.tile([1, N], F32)
        c1 = cp.tile([128, 3, R], BF16)
        clg = cp.tile([R, 18], F32)
        x = cp.tile([R, N], F32)
        pexp = cp.tile([R, N], F32)
        escr = cp.tile([R, N], F32)
        z = cp.tile([R, 1], F32)
        e = cp.tile([R, 1], F32)
        lz = cp.tile([R, 1], F32)
        le = cp.tile([R, 1], F32)
        tnll = cp.tile([R, 1], F32)
        cexp = cp.tile([R, 18], F32)
        cz = cp.tile([R, 1], F32)
        cscr = cp.tile([R, 18], F32)
        sl = cp.tile([R, 1], F32)
        lcz = cp.tile([R, 1], F32)
        cev = cp.tile([R, 1], F32)

        # ---------- preamble: A.T cols 0:NH, Bb ----------
        with tc.tile_pool(name=f"pre_ps{sfx}", bufs=1, space="PSUM") as pp:
            for co in range(HC):
                pa = pp.tile([128, NH], F32, tag="at", name=f"pa_{co}{sfx}",
                             bufs=2)
                for ci in range(HC):
                    nc.tensor.matmul(
                        out=pa[:],
                        lhsT=waTc[co][:, ci, :],
                        rhs=rT[:, ci, 0:NH],
                        start=(ci == 0),
                        stop=(ci == HC - 1),
                    )
                nc.scalar.copy(out=at_sb[:, co, 0:NH], in_=pa[:])
            for co in range(HC):
                pb = pp.tile([128, R], F32, tag="bb", name=f"pb_{co}{sfx}",
                             bufs=2)
                for ci in range(HC):
                    nc.tensor.matmul(
                        out=pb[:],
                        lhsT=wbT_sb[:, ci, co * 128 : (co + 1) * 128],
                        rhs=rTl[:, ci, :],
                        start=(ci == 0),
                        stop=(ci == HC - 1),
                    )
                nc.vector.tensor_scalar(
                    out=bb_sb[:, co, :],
                    in0=pb[:],
                    scalar1=b1c_sb[:, co : co + 1],
                    scalar2=None,
                    op0=OP.add,
                )

        # ---------- main loop + interleaved epilogue ----------
        with (
            tc.tile_pool(name=f"lp_sb{sfx}", bufs=1) as lsb,
            tc.tile_pool(name=f"lp_ps{sfx}", bufs=2, space="PSUM") as lps,
            tc.tile_pool(name=f"sr_ps{sfx}", bufs=1, space="PSUM") as sps,
            tc.tile_pool(name=f"ep_ps{sfx}", bufs=1, space="PSUM") as eps,
        ):
            def ep_tile(nm):
                return eps.tile([128, N], F32, tag="ep", name=f"{nm}{sfx}")

            def at_h2(co):
                pa = ep_tile(f"pa2_{co}")
                for ci in range(HC):
                    nc.tensor.matmul(
                        out=pa[:, 0 : N - NH],
                        lhsT=waTc[co][:, ci, :],
                        rhs=rT[:, ci, NH:N],
                        start=(ci == 0),
                        stop=(ci == HC - 1),
                    )
                nc.scalar.copy(out=at_sb[:, co, NH:N], in_=pa[:, 0 : N - NH])

            def ms1_co(co):
                pm = ep_tile(f"pm_{co}")
                for ci in range(HC):
                    nc.tensor.matmul(
                        out=pm[:],
                        lhsT=wm1T_sb[:, ci, co * 128 : (co + 1) * 128],
                        rhs=rT[:, ci, :],
                        start=(ci == 0),
                        stop=(ci == HC - 1),
                    )
                nc.scalar.activation(
                    out=ms1[:, co, :], in_=pm[:], func=AF.Relu,
                    bias=bm1c_sb[:, co : co + 1],
                )

            def ms2_co(co):
                sz = (128, 64)[co]
                pm2 = ep_tile(f"pm2_{co}")
                for ci in range(3):
                    nc.tensor.matmul(
                        out=pm2[:sz, :],
                        lhsT=wm2T_sb[:, ci, co * 128 : co * 128 + sz],
                        rhs=ms1[:, ci, :],
                        start=(ci == 0),
                        stop=(ci == 2),
                    )
                nc.scalar.activation(
                    out=ms2[:sz, co, :], in_=pm2[:sz, :], func=AF.Relu,
                    bias=bm2c_sb[:sz, co : co + 1],
                )

            def ms3_mskms():
                pms = ep_tile("pms")
                nc.tensor.matmul(
                    out=pms[0:1, :], lhsT=wm3c_sb[:, 0:1], rhs=ms2[:, 0, :],
                    start=True, stop=False,
                )
                nc.tensor.matmul(
                    out=pms[0:1, :], lhsT=wm3c_sb[:64, 1:2], rhs=ms2[:64, 1, :],
                    start=False, stop=True,
                )
                nc.vector.tensor_copy(out=ms_sb[:], in_=pms[0:1, :])
                pbc = ep_tile("pbc")
                nc.tensor.matmul(
                    out=pbc[0:R, :], lhsT=one1[:], rhs=ms_sb[:],
                    start=True, stop=True,
                )
                nc.vector.tensor_tensor(
                    out=mskms[:], in0=pbc[0:R, :], in1=maskb_sb[:], op=OP.add
                )

            def char1_co(co):
                pc = ep_tile(f"pc_{co}")
                for ci in range(HC):
                    nc.tensor.matmul(
                        out=pc[:, 0:R],
                        lhsT=wc1T_sb[:, ci, co * 128 : (co + 1) * 128],
                        rhs=rTl[:, ci, :],
                        start=(ci == 0),
                        stop=(ci == HC - 1),
                    )
                nc.scalar.activation(
                    out=c1[:, co, :], in_=pc[:, 0:R], func=AF.Relu,
                    bias=bc1c_sb[:, co : co + 1],
                )

            def char_lg():
                plg = ep_tile("plg")
                for co in range(3):
                    nc.tensor.matmul(
                        out=plg[0:R, 0:18], lhsT=c1[:, co, :],
                        rhs=wc2T_sb[:, co, :],
                        start=(co == 0), stop=False,
                    )
                nc.tensor.matmul(
                    out=plg[0:R, 0:18], lhsT=one1[:], rhs=bc2r_sb[:],
                    start=False, stop=True,
                )
                nc.vector.tensor_copy(out=clg[:], in_=plg[0:R, 0:18])

            def char_sm_a():
                # scores are O(1); no row-max subtraction needed in fp32
                nc.scalar.activation(
                    out=cexp[:], in_=clg[:], func=AF.Exp, accum_out=cz[:],
                )

            def char_sm_b():
                nc.vector.tensor_tensor(
                    out=cscr[:], in0=clg[:], in1=oneh_sb[:], op=OP.mult
                )
                nc.vector.tensor_reduce(
                    out=sl[:], in_=cscr[:], axis=mybir.AxisListType.X, op=OP.add
                )

            def nll_exp(r0, r1):
                s = slice(r0, r1)
                nc.vector.tensor_tensor(
                    out=x[s, :], in0=sblk[s, :], in1=mskms[s, :], op=OP.add
                )
                nc.scalar.activation(
                    out=pexp[s, :], in_=x[s, :], func=AF.Exp,
                    accum_out=z[s, :],
                )
                nc.vector.tensor_tensor(
                    out=escr[s, :], in0=pexp[s, :], in1=multb_sb[s, :],
                    op=OP.mult,
                )
                nc.vector.tensor_reduce(
                    out=e[s, :], in_=escr[s, :], axis=mybir.AxisListType.X,
                    op=OP.add,
                )

            def ln_block():
                # all Ln ops back-to-back: one activation-table switch
                nc.scalar.activation(out=lz[:], in_=z[:], func=AF.Ln)
                nc.scalar.activation(out=le[:], in_=e[:], func=AF.Ln)
                nc.scalar.activation(out=lcz[:], in_=cz[:], func=AF.Ln)
                nc.vector.tensor_tensor(
                    out=tnll[:], in0=lz[:], in1=le[:], op=OP.subtract
                )
                nc.vector.tensor_tensor(
                    out=cev[:], in0=lcz[:], in1=sl[:], op=OP.subtract
                )

            nll2_at = _last_batch_with_row_ge(32) + 1
            extras = {
                0: [lambda: at_h2(0), lambda: at_h2(1), lambda: at_h2(2)],
                1: [lambda: at_h2(3), lambda: at_h2(4), lambda: at_h2(5)],
                2: [lambda: ms1_co(0)],
                3: [lambda: ms1_co(1)],
                4: [lambda: ms1_co(2)],
                5: [lambda: ms2_co(0)],
                6: [lambda: ms2_co(1)],
                7: [ms3_mskms],
                8: [lambda: char1_co(0)],
                9: [lambda: char1_co(1)],
                10: [lambda: char1_co(2)],
                11: [char_lg],
                12: [char_sm_a],
                13: [char_sm_b],
                nll2_at: [lambda: nll_exp(32, R)],
            }

            pend = None  # (h2sa tile, F, segs, idx) awaiting W3 reduce

            def flush_pend():
                h2sa, F, segs, bi = pend
                sr = sps.tile([1, FMAX], F32, tag="srow", name=f"sr_{bi}{sfx}",
                              bufs=1)
                if FP8:
                    nc.tensor.matmul(
                        out=sr[:, :F], lhsT=w3p_sb[:, :, 0:1],
                        rhs=h2sa[:, 0:2, :F],
                        start=True, stop=False, perf_mode=PM.DoubleRow,
                    )
                    nc.tensor.matmul(
                        out=sr[:, :F], lhsT=w3s_sb[:, 0:1],
                        rhs=h2sa[:, 2, :F],
                        start=False, stop=True,
                    )
                else:
                    for hb in range(3):
                        nc.tensor.matmul(
                            out=sr[:, :F],
                            lhsT=w3c_sb[:, hb : hb + 1],
                            rhs=h2sa[:, hb, :F],
                            start=(hb == 0),
                            stop=(hb == 2),
                        )
                srow = lsb.tile([1, FMAX], F32, tag="srow_sb",
                                name=f"srow_{bi}{sfx}", bufs=2)
                nc.scalar.mul(out=srow[:, :F], in_=sr[:, :F],
                              mul=(1.0 / WSCALE) if FP8 else 1.0)
                for (k0, G, C, off) in segs:
                    nc.sync.dma_start(
                        out=sblk[k0 : k0 + G, 0:C],
                        in_=srow[0:1, off : off + G * C],
                    )

            H1DT = FP8DT if FP8 else BF16
            for pass_ in range(mainloop_reps):
                for bi, segs in enumerate(BATCHES):
                    F = _batch_F(segs)
                    h1 = lsb.tile([128, HC, FMAX], H1DT, tag="h1",
                                  name=f"h1_{pass_}_{bi}{sfx}", bufs=3)
                    for c in range(HC):
                        for (k0, G, C, off) in segs:
                            for g in range(G):
                                nc.vector.tensor_scalar(
                                    out=h1[:, c, off + g * C : off + (g + 1) * C],
                                    in0=at_sb[:, c, 0:C],
                                    scalar1=bb_sb[:, c, k0 + g : k0 + g + 1],
                                    scalar2=0.0,
                                    op0=OP.add,
                                    op1=OP.max,
                                )
                    h2sa = lsb.tile([128, 3, FMAX], H1DT, tag="h2sa",
                                    name=f"hs_{pass_}_{bi}{sfx}", bufs=2)
                    for hb in range(3):
                        ph = lps.tile([128, FMAX], F32, tag=f"h2_{hb}",
                                      name=f"ph_{pass_}_{bi}_{hb}{sfx}")
                        if FP8:
                            for kp in range(3):
                                nc.tensor.matmul(
                                    out=ph[:, :F],
                                    lhsT=w2p_sb[:, kp, :,
                                                hb * 128 : (hb + 1) * 128],
                                    rhs=h1[:, 2 * kp : 2 * kp + 2, 0:F],
                                    start=(kp == 0),
                                    stop=(kp == 2),
                                    perf_mode=PM.DoubleRow,
                                )
                        else:
                            for c in range(HC):
                                nc.tensor.matmul(
                                    out=ph[:, :F],
                                    lhsT=w2T_sb[:, c, hb * 128 : (hb + 1) * 128],
                                    rhs=h1[:, c, 0:F],
                                    start=(c == 0),
                                    stop=(c == HC - 1),
                                )
                        nc.scalar.activation(
                            out=h2sa[:, hb, :F], in_=ph[:, :F], func=AF.Relu,
                            bias=b2c_sb[:, hb : hb + 1],
                            scale=(1.0 / WSCALE) if FP8 else 1.0,
                        )
                    if pend is not None:
                        flush_pend()
                    pend = (h2sa, F, segs, f"{pass_}_{bi}")
                    if pass_ == 0:
                        for fn in extras.get(bi, []):
                            fn()
            flush_pend()

            # ---------- tail ----------
            nll_exp(0, 32)
            ln_block()
            pl = ep_tile("pl")
            nc.tensor.matmul(
                out=pl[0:1, 0:1], lhsT=tnll[:, 0:1], rhs=wnll_sb[:],
                start=True, stop=False,
            )
            nc.tensor.matmul(
                out=pl[0:1, 0:1], lhsT=cev[:, 0:1], rhs=wch_sb[:],
                start=False, stop=True,
            )
            lout = cp.tile([1, 1], F32)
            nc.vector.tensor_copy(out=lout[:], in_=pl[0:1, 0:1])
            nc.sync.dma_start(out=T["loss"].ap(), in_=lout[:])


def _build_program(repeat=1, mainloop_reps=1):
    nc = bacc.Bacc(
        "TRN2", target_bir_lowering=False, debug=False, enable_asserts=False
    )
    T = _declare_inputs(nc)
    with tile.TileContext(nc) as tc:
        for rep in range(repeat):
            _emit_core(nc, tc, T, f"_r{rep}" if repeat > 1 else "",
                       mainloop_reps=mainloop_reps)
    nc.compile()
    return nc


def _chunk_cols(w):
    """[K, O] -> [128, K//128, O]  (partition-chunked contraction dim)."""
    k, o = w.shape
    return np.ascontiguousarray(w.reshape(k // 128, 128, o).transpose(1, 0, 2))


def _chunk_vec(v, ncol):
    """[C] -> [128, ncol] column-chunks (zero padded)."""
    out = np.zeros((128, ncol), np.float32)
    for c in range(ncol):
        seg = v[c * 128 : (c + 1) * 128]
        out[: len(seg), c] = seg
    return out


def _prep_in_maps(inputs):
    bf = ml_dtypes.bfloat16

    seq = np.asarray(inputs["sequence_output"], np.float32)
    spk = np.asarray(inputs["speaker_emb"], np.float32)
    dummy = np.asarray(inputs["dummy_emb"], np.float32)

    seg = np.asarray(inputs["mentions_seg"]).astype(np.int64)
    mstart = np.asarray(inputs["mention_start"]).astype(np.int64)
    mend = np.asarray(inputs["mention_end"]).astype(np.int64)
    sid = np.asarray(inputs["speaker_ids"]).astype(np.int64)[seg, mstart]
    mention_reps = seq[seg, mstart] + seq[seg, mend] + spk[sid]  # [M, H] f32
    all_reps = np.concatenate([dummy, mention_reps], axis=0)     # [N, H]
    # rT[p, c, m] = all_reps[m, c*128+p]
    rT_np = np.ascontiguousarray(
        all_reps.reshape(N, HC, 128).transpose(2, 1, 0)
    ).astype(bf)                                                 # [128, HC, N]

    W_pair1 = np.asarray(inputs["W_pair1"], np.float32)
    waT = _chunk_cols(np.ascontiguousarray(W_pair1[:, :H].T)).astype(bf)
    wbT = _chunk_cols(np.ascontiguousarray(W_pair1[:, H:].T)).astype(bf)
    w2T = _chunk_cols(
        np.ascontiguousarray(np.asarray(inputs["W_pair2"], np.float32).T)
    )                                                            # [128, 6, 384] f32
    w3c = _chunk_vec(np.asarray(inputs["W_pair3"], np.float32)[0], 3)
    if FP8:
        f8 = mybir.dt.np(FP8DT)
        w2p = np.ascontiguousarray(
            (w2T * WSCALE).reshape(128, 3, 2, H // 2)
        ).astype(f8)                                             # paired k-chunks
        w3p = np.zeros((128, 2, 16), np.float32)
        w3p[:, 0, 0] = w3c[:, 0] * WSCALE
        w3p[:, 1, 0] = w3c[:, 1] * WSCALE
        w3p = w3p.astype(f8)
        w3s = (w3c[:, 2:3] * WSCALE).astype(f8)
    w2T = w2T.astype(bf)
    w3c = w3c.astype(bf)
    b1c = _chunk_vec(np.asarray(inputs["b_pair1"], np.float32), HC)
    b2c = _chunk_vec(np.asarray(inputs["b_pair2"], np.float32), 3)
    wm1T = _chunk_cols(
        np.ascontiguousarray(np.asarray(inputs["W_m1"], np.float32).T)
    ).astype(bf)
    bm1c = _chunk_vec(np.asarray(inputs["b_m1"], np.float32), 3)
    wm2T = _chunk_cols(
        np.ascontiguousarray(np.asarray(inputs["W_m2"], np.float32).T)
    ).astype(bf)
    bm2c = _chunk_vec(np.asarray(inputs["b_m2"], np.float32), 2)
    wm3c = _chunk_vec(np.asarray(inputs["W_m3"], np.float32)[0], 2).astype(bf)
    wc1T = _chunk_cols(
        np.ascontiguousarray(np.asarray(inputs["W_c1"], np.float32).T)
    ).astype(bf)
    bc1c = _chunk_vec(np.asarray(inputs["b_c1"], np.float32), 3)
    wc2T = _chunk_cols(
        np.ascontiguousarray(np.asarray(inputs["W_c2"], np.float32).T)
    ).astype(bf)
    bc2r = np.asarray(inputs["b_c2"], np.float32).reshape(1, 18)

    link_first = np.asarray(inputs["link_first"]).astype(np.int64)
    link_second = np.asarray(inputs["link_second"]).astype(np.int64)
    label = np.asarray(inputs["character_label"]).astype(np.int64)

    mult = np.zeros((N, N), np.float32)
    np.add.at(mult, (link_second, link_first), 1.0)
    has_link = mult.sum(axis=1) > 0
    wnll_full = ((np.arange(N) >= 1) & has_link).astype(np.float32)
    mult[~has_link, 0] = 1.0  # keep log(E) finite; weight is 0 there

    mask_full = np.where(
        np.arange(N)[None, :] >= np.arange(N)[:, None], np.float32(NEG), 0.0
    ).astype(np.float32)

    oneh_full = np.zeros((N, 18), np.float32)
    wch_full = np.zeros(N, np.float32)
    oneh_full[np.arange(1, N), label] = 1.0
    wch_full[1:] = 1.0

    shared = dict(
        rT_in=rT_np,
        wbT=wbT, b1c=b1c, b2c=b2c,
        wm1T=wm1T, bm1c=bm1c, wm2T=wm2T, bm2c=bm2c, wm3c=wm3c,
        wc1T=wc1T, bc1c=bc1c, wc2T=wc2T, bc2r=bc2r,
    )
    if FP8:
        shared.update(w2p=w2p, w3p=w3p, w3s=w3s)
    else:
        shared.update(w2T=w2T, w3c=w3c)
    for co in range(HC):
        shared[f"waT{co}"] = np.ascontiguousarray(
            waT[:, :, co * 128 : (co + 1) * 128]
        )
    in_maps = []
    for d in range(NC_):
        rows = np.arange(R) * NC_ + d      # modulo sharding: row 8k+d
        m = dict(shared)
        m["rTl_in"] = np.ascontiguousarray(rT_np[:, :, rows])
        m["maskb"] = np.ascontiguousarray(mask_full[rows])
        if d == 0:
            # Global row 0 is fully masked; without row-max subtraction its
            # Z would be exactly 0 (ln -> -inf). Unmask its [0,0] entry:
            # mult[0,0]=1 and wnll[0]=0 make the row contribute exactly 0.
            m["maskb"][0, 0] = 0.0
        m["multb"] = np.ascontiguousarray(mult[rows])
        m["wnll"] = np.ascontiguousarray(wnll_full[rows]).reshape(R, 1)
        m["oneh"] = np.ascontiguousarray(oneh_full[rows])
        m["wch"] = np.ascontiguousarray(wch_full[rows]).reshape(R, 1)
        in_maps.append(m)
    return in_maps


def kernel(**inputs):
    global LAST_RESULT
    in_maps = _prep_in_maps(inputs)

    if "nc" not in _CACHE:
        _CACHE["nc"] = _build_program()
    nc = _CACHE["nc"]

    res = run_bass_kernel_spmd(nc, in_maps, core_ids=list(range(NC_)))
    LAST_RESULT = res
    total = np.float32(0.0)
    for d in range(NC_):
        total += np.float32(res.results[d]["loss"][0, 0])
    return np.asarray(total, dtype=np.float32)


if __name__ == "__main__":
    import reference

    inputs = {k: np.asarray(v) for k, v in reference.setup_inputs().items()}
    out = kernel(**inputs)
    print("kernel out:", out)


# revision 39
# speedup vs baseline: 1.2657x; 1.1844x over previous
"""Trainium2 Bass kernel for nn_JointLearningModel (coref-style joint model).

Sharding: the 384x384 pair grid is split by rows across 8 NeuronCores,
row i -> core i%8 (modulo sharding). Only the lower triangle j < i is
computed: with modulo sharding, local row k on any core has global index
8k+d (d<8), so a core-independent static column extent C_k =
roundup(8k+7, 32) covers every core's true extent and the per-core PE
work is identical (perfect balance). Columns beyond the true extent are
killed by the causal mask (-1e4) before the row softmax, which
underflows to exactly 0 in fp32.

Mention representations are gathered and transposed on the host (the
sharding hint treats all_mention_representations as replicated inputs);
params replicated; the scalar loss is summed on host across cores.

Schedule: weight DMAs are split across the SP and Activation HWDGE
queues; A.T is computed in two column halves so the main loop starts
~2.5us in; the mention-score MLP, character head, and softmax epilogue
are interleaved into the main loop as PE filler; the per-batch W3 score
reduction is deferred by one batch so PE never waits on the scalar
engine's relu output.
"""

import numpy as np
import ml_dtypes

import concourse.bass as bass
import concourse.mybir as mybir
import concourse.tile as tile
from concourse import bacc
from concourse.bass_utils import run_bass_kernel_spmd

F32 = mybir.dt.float32
BF16 = mybir.dt.bfloat16
FP8DT = mybir.dt.float8e4
PM = mybir.MatmulPerfMode
AF = mybir.ActivationFunctionType
OP = mybir.AluOpType

B, L, H, M = 8, 512, 768, 383
N = M + 1          # 384 rows/cols of the pair grid
NC_ = 8            # cores
R = N // NC_       # 48 rows per core
HC = H // 128      # 6 k-chunks of the hidden dim
NEG = -10000.0
FMAX = 512         # PSUM bank capacity in fp32 elements per partition
NH = 192           # A.T column half size

_CACHE = {}
LAST_RESULT = None

FP8 = True          # fp8e4 DoubleRow for the pair-MLP h2/w3 matmuls
WSCALE = 64.0       # fp8 weight pre-scale, compensated in the relu/copy


def _extent(k):
    """Static column extent for local row k (covers 8k+d for all d<8)."""
    return min(N, 32 * ((8 * k + 7 + 31) // 32))


def _batch_plan():
    """Pack rows into batches of segments with total F <= FMAX.

    Rows 0..11 (tiny extents) pack ascending; then each big row (desc
    from 47) pairs with the smallest-index unassigned rows that fit its
    remaining capacity. For this problem the packing is perfect: 20
    batches, 18 of them exactly F=512. Each batch is a list of segments
    (k0, G, C, off): G consecutive rows sharing extent C at offset off.
    """
    Cs = [_extent(k) for k in range(R)]

    def to_segments(rows):
        segs = []
        i = 0
        off = 0
        while i < len(rows):
            j = i
            while (j + 1 < len(rows) and rows[j + 1] == rows[j] + 1
                   and Cs[rows[j + 1]] == Cs[rows[i]]):
                j += 1
            g = j - i + 1
            segs.append((rows[i], g, Cs[rows[i]], off))
            off += g * Cs[rows[i]]
            i = j + 1
        return segs

    batches = []
    # early ascending fill over rows 0..11
    early = list(range(12))
    cur, cap = [], FMAX
    for k in early:
        if Cs[k] > cap:
            batches.append(to_segments(cur))
            cur, cap = [], FMAX
        cur.append(k)
        cap -= Cs[k]
    if cur:
        batches.append(to_segments(cur))
    # big rows descending, padded with smallest unassigned rows
    unassigned = list(range(12, R))
    while unassigned:
        big = unassigned.pop()          # largest index = largest extent
        cur, cap = [big], FMAX - Cs[big]
        while unassigned and Cs[unassigned[0]] <= cap:
            k = unassigned.pop(0)
            cur.append(k)
            cap -= Cs[k]
        batches.append(to_segments(sorted(cur)))
    assert sorted(k for b in batches for (k0, g, c, off) in b
                  for k in range(k0, k0 + g)) == list(range(R))
    return batches


BATCHES = _batch_plan()


def _batch_F(b):
    return sum(g * c for (k0, g, c, off) in b)


def _last_batch_with_row_ge(r):
    last = 0
    for i, b in enumerate(BATCHES):
        if any(k0 + g > r for (k0, g, c, off) in b):
            last = i
    return last


def _declare_inputs(nc):
    def din(name, shape, dt):
        return nc.dram_tensor(name, list(shape), dt, kind="ExternalInput")

    T = {}
    T["rT_in"] = din("rT_in", [128, HC, N], BF16)
    T["rTl_in"] = din("rTl_in", [128, HC, R], BF16)
    # waT split by output chunk for early-start loads
    for co in range(HC):
        T[f"waT{co}"] = din(f"waT{co}", [128, HC, 128], BF16)
    T["wbT"] = din("wbT", [128, HC, H], BF16)
    if FP8:
        T["w2p"] = din("w2p", [128, 3, 2, H // 2], FP8DT)
        T["w3p"] = din("w3p", [128, 2, 16], FP8DT)
        T["w3s"] = din("w3s", [128, 1], FP8DT)
    else:
        T["w2T"] = din("w2T", [128, HC, H // 2], BF16)
        T["w3c"] = din("w3c", [128, 3], BF16)
    T["b1c"] = din("b1c", [128, HC], F32)
    T["b2c"] = din("b2c", [128, 3], F32)
    T["wm1T"] = din("wm1T", [128, HC, H // 2], BF16)
    T["bm1c"] = din("bm1c", [128, 3], F32)
    T["wm2T"] = din("wm2T", [128, 3, H // 4], BF16)
    T["bm2c"] = din("bm2c", [128, 2], F32)
    T["wm3c"] = din("wm3c", [128, 2], BF16)
    T["wc1T"] = din("wc1T", [128, HC, H // 2], BF16)
    T["bc1c"] = din("bc1c", [128, 3], F32)
    T["wc2T"] = din("wc2T", [128, 3, 18], BF16)
    T["bc2r"] = din("bc2r", [1, 18], F32)
    T["maskb"] = din("maskb", [R, N], F32)
    T["multb"] = din("multb", [R, N], F32)
    T["wnll"] = din("wnll", [R, 1], F32)
    T["oneh"] = din("oneh", [R, 18], F32)
    T["wch"] = din("wch", [R, 1], F32)
    T["loss"] = nc.dram_tensor("loss", [1, 1], F32, kind="ExternalOutput")
    return T


def _emit_core(nc, tc, T, sfx, mainloop_reps=1, sink=None):
    with tc.tile_pool(name=f"const{sfx}", bufs=1) as cp:
        def load(name, h, eng):
            t = cp.tile(list(h.shape), h.dtype, name=f"{name}{sfx}")
            eng.dma_start(out=t[:], in_=h.ap())
            return t

        # queue 1 (SP): what the first PE ops need, in order
        rT = load("rT", T["rT_in"], nc.sync)
        waTc = [load(f"waTc{co}", T[f"waT{co}"], nc.sync) for co in range(HC)]
        if FP8:
            w2p_sb = load("w2p_sb", T["w2p"], nc.sync)
            w3p_sb = load("w3p_sb", T["w3p"], nc.sync)
            w3s_sb = load("w3s_sb", T["w3s"], nc.sync)
        else:
            w2T_sb = load("w2T_sb", T["w2T"], nc.sync)
            w3c_sb = load("w3c_sb", T["w3c"], nc.sync)
        b1c_sb = load("b1c_sb", T["b1c"], nc.sync)
        b2c_sb = load("b2c_sb", T["b2c"], nc.sync)
        # queue 2 (Activation): everything else
        rTl = load("rTl", T["rTl_in"], nc.scalar)
        wbT_sb = load("wbT_sb", T["wbT"], nc.scalar)
        wm1T_sb = load("wm1T_sb", T["wm1T"], nc.scalar)
        bm1c_sb = load("bm1c_sb", T["bm1c"], nc.scalar)
        wm2T_sb = load("wm2T_sb", T["wm2T"], nc.scalar)
        bm2c_sb = load("bm2c_sb", T["bm2c"], nc.scalar)
        wm3c_sb = load("wm3c_sb", T["wm3c"], nc.scalar)
        wc1T_sb = load("wc1T_sb", T["wc1T"], nc.scalar)
        bc1c_sb = load("bc1c_sb", T["bc1c"], nc.scalar)
        wc2T_sb = load("wc2T_sb", T["wc2T"], nc.scalar)
        bc2r_sb = load("bc2r_sb", T["bc2r"], nc.scalar)
        maskb_sb = load("maskb_sb", T["maskb"], nc.scalar)
        multb_sb = load("multb_sb", T["multb"], nc.scalar)
        wnll_sb = load("wnll_sb", T["wnll"], nc.scalar)
        oneh_sb = load("oneh_sb", T["oneh"], nc.scalar)
        wch_sb = load("wch_sb", T["wch"], nc.scalar)

        one1 = cp.tile([1, R], F32)
        nc.vector.memset(one1[:], 1.0)

        at_sb = cp.tile([128, HC, N], BF16)    # A.T   (bf16)
        bb_sb = cp.tile([128, HC, R], F32)     # Bm.T + b1, local rows
        sblk = cp.tile([R, N], F32)            # assembled pair scores
        nc.vector.memset(sblk[:], 0.0)
        mskms = cp.tile([R, N], F32)           # mask + ms[j] broadcast
        ms1 = cp.tile([128, 3, N], BF16)
        ms2 = cp.tile([128, 2, N], BF16)
        ms_sb = cp.tile([1, N], F32)
        c1 = cp.tile([128, 3, R], BF16)
        clg = cp.tile([R, 18], F32)
        x = cp.tile([R, N], F32)
        pexp = cp.tile([R, N], F32)
        escr = cp.tile([R, N], F32)
        z = cp.tile([R, 1], F32)
        e = cp.tile([R, 1], F32)
        lz = cp.tile([R, 1], F32)
        le = cp.tile([R, 1], F32)
        tnll = cp.tile([R, 1], F32)
        cexp = cp.tile([R, 18], F32)
        cz = cp.tile([R, 1], F32)
        cscr = cp.tile([R, 18], F32)
        sl = cp.tile([R, 1], F32)
        lcz = cp.tile([R, 1], F32)
        cev = cp.tile([R, 1], F32)

        # ---------- preamble: A.T cols 0:NH, Bb ----------
        with tc.tile_pool(name=f"pre_ps{sfx}", bufs=1, space="PSUM") as pp:
            for co in range(HC):
                pa = pp.tile([128, NH], F32, tag="at", name=f"pa_{co}{sfx}",
                             bufs=2)
                for ci in range(HC):
                    nc.tensor.matmul(
                        out=pa[:],
                        lhsT=waTc[co][:, ci, :],
                        rhs=rT[:, ci, 0:NH],
                        start=(ci == 0),
                        stop=(ci == HC - 1),
                    )
                nc.scalar.copy(out=at_sb[:, co, 0:NH], in_=pa[:])
            for co in range(HC):
                pb = pp.tile([128, R], F32, tag="bb", name=f"pb_{co}{sfx}",
                             bufs=2)
                for ci in range(HC):
                    nc.tensor.matmul(
                        out=pb[:],
                        lhsT=wbT_sb[:, ci, co * 128 : (co + 1) * 128],
                        rhs=rTl[:, ci, :],
                        start=(ci == 0),
                        stop=(ci == HC - 1),
                    )
                nc.vector.tensor_scalar(
                    out=bb_sb[:, co, :],
                    in0=pb[:],
                    scalar1=b1c_sb[:, co : co + 1],
                    scalar2=None,
                    op0=OP.add,
                )

        # ---------- main loop + interleaved epilogue ----------
        with (
            tc.tile_pool(name=f"lp_sb{sfx}", bufs=1) as lsb,
            tc.tile_pool(name=f"lp_ps{sfx}", bufs=2, space="PSUM") as lps,
            tc.tile_pool(name=f"sr_ps{sfx}", bufs=1, space="PSUM") as sps,
            tc.tile_pool(name=f"ep_ps{sfx}", bufs=1, space="PSUM") as eps,
        ):
            def ep_tile(nm):
                return eps.tile([128, N], F32, tag="ep", name=f"{nm}{sfx}")

            def at_h2(co):
                pa = ep_tile(f"pa2_{co}")
                for ci in range(HC):
                    nc.tensor.matmul(
                        out=pa[:, 0 : N - NH],
                        lhsT=waTc[co][:, ci, :],
                        rhs=rT[:, ci, NH:N],
                        start=(ci == 0),
                        stop=(ci == HC - 1),
                    )
                nc.scalar.copy(out=at_sb[:, co, NH:N], in_=pa[:, 0 : N - NH])

            def ms1_co(co):
                pm = ep_tile(f"pm_{co}")
                for ci in range(HC):
                    nc.tensor.matmul(
                        out=pm[:],
                        lhsT=wm1T_sb[:, ci, co * 128 : (co + 1) * 128],
                        rhs=rT[:, ci, :],
                        start=(ci == 0),
                        stop=(ci == HC - 1),
                    )
                nc.scalar.activation(
                    out=ms1[:, co, :], in_=pm[:], func=AF.Relu,
                    bias=bm1c_sb[:, co : co + 1],
                )

            def ms2_co(co):
                sz = (128, 64)[co]
                pm2 = ep_tile(f"pm2_{co}")
                for ci in range(3):
                    nc.tensor.matmul(
                        out=pm2[:sz, :],
                        lhsT=wm2T_sb[:, ci, co * 128 : co * 128 + sz],
                        rhs=ms1[:, ci, :],
                        start=(ci == 0),
                        stop=(ci == 2),
                    )
                nc.scalar.activation(
                    out=ms2[:sz, co, :], in_=pm2[:sz, :], func=AF.Relu,
                    bias=bm2c_sb[:sz, co : co + 1],
                )

            def ms3_mskms():
                pms = ep_tile("pms")
                nc.tensor.matmul(
                    out=pms[0:1, :], lhsT=wm3c_sb[:, 0:1], rhs=ms2[:, 0, :],
                    start=True, stop=False,
                )
                nc.tensor.matmul(
                    out=pms[0:1, :], lhsT=wm3c_sb[:64, 1:2], rhs=ms2[:64, 1, :],
                    start=False, stop=True,
                )
                nc.vector.tensor_copy(out=ms_sb[:], in_=pms[0:1, :])
                pbc = ep_tile("pbc")
                nc.tensor.matmul(
                    out=pbc[0:R, :], lhsT=one1[:], rhs=ms_sb[:],
                    start=True, stop=True,
                )
                nc.vector.tensor_tensor(
                    out=mskms[:], in0=pbc[0:R, :], in1=maskb_sb[:], op=OP.add
                )

            def char1_co(co):
                pc = ep_tile(f"pc_{co}")
                for ci in range(HC):
                    nc.tensor.matmul(
                        out=pc[:, 0:R],
                        lhsT=wc1T_sb[:, ci, co * 128 : (co + 1) * 128],
                        rhs=rTl[:, ci, :],
                        start=(ci == 0),
                        stop=(ci == HC - 1),
                    )
                nc.scalar.activation(
                    out=c1[:, co, :], in_=pc[:, 0:R], func=AF.Relu,
                    bias=bc1c_sb[:, co : co + 1],
                )

            def char_lg():
                plg = ep_tile("plg")
                for co in range(3):
                    nc.tensor.matmul(
                        out=plg[0:R, 0:18], lhsT=c1[:, co, :],
                        rhs=wc2T_sb[:, co, :],
                        start=(co == 0), stop=False,
                    )
                nc.tensor.matmul(
                    out=plg[0:R, 0:18], lhsT=one1[:], rhs=bc2r_sb[:],
                    start=False, stop=True,
                )
                nc.vector.tensor_copy(out=clg[:], in_=plg[0:R, 0:18])

            def char_sm_a():
                # scores are O(1); no row-max subtraction needed in fp32
                nc.scalar.activation(
                    out=cexp[:], in_=clg[:], func=AF.Exp, accum_out=cz[:],
                )

            def char_sm_b():
                nc.vector.tensor_tensor(
                    out=cscr[:], in0=clg[:], in1=oneh_sb[:], op=OP.mult
                )
                nc.vector.tensor_reduce(
                    out=sl[:], in_=cscr[:], axis=mybir.AxisListType.X, op=OP.add
                )

            def nll_exp(r0, r1):
                s = slice(r0, r1)
                nc.vector.tensor_tensor(
                    out=x[s, :], in0=sblk[s, :], in1=mskms[s, :], op=OP.add
                )
                nc.scalar.activation(
                    out=pexp[s, :], in_=x[s, :], func=AF.Exp,
                    accum_out=z[s, :],
                )
                nc.vector.tensor_tensor(
                    out=escr[s, :], in0=pexp[s, :], in1=multb_sb[s, :],
                    op=OP.mult,
                )
                nc.vector.tensor_reduce(
                    out=e[s, :], in_=escr[s, :], axis=mybir.AxisListType.X,
                    op=OP.add,
                )

            def ln_block():
                # all Ln ops back-to-back: one activation-table switch
                nc.scalar.activation(out=lz[:], in_=z[:], func=AF.Ln)
                nc.scalar.activation(out=le[:], in_=e[:], func=AF.Ln)
                nc.scalar.activation(out=lcz[:], in_=cz[:], func=AF.Ln)
                nc.vector.tensor_tensor(
                    out=tnll[:], in0=lz[:], in1=le[:], op=OP.subtract
                )
                nc.vector.tensor_tensor(
                    out=cev[:], in0=lcz[:], in1=sl[:], op=OP.subtract
                )

            nll2_at = _last_batch_with_row_ge(32) + 1
            extras = {
                0: [lambda: at_h2(0), lambda: at_h2(1), lambda: at_h2(2)],
                1: [lambda: at_h2(3), lambda: at_h2(4), lambda: at_h2(5)],
                2: [lambda: ms1_co(0)],
                3: [lambda: ms1_co(1)],
                4: [lambda: ms1_co(2)],
                5: [lambda: ms2_co(0)],
                6: [lambda: ms2_co(1)],
                7: [ms3_mskms],
                8: [lambda: char1_co(0)],
                9: [lambda: char1_co(1)],
                10: [lambda: char1_co(2)],
                11: [char_lg],
                12: [char_sm_a],
                13: [char_sm_b],
                nll2_at: [lambda: nll_exp(32, R)],
            }

            pend = None  # (h2sa tile, F, segs, idx) awaiting W3 reduce

            def flush_pend():
                h2sa, F, segs, bi = pend
                sr = sps.tile([1, FMAX], F32, tag="srow", name=f"sr_{bi}{sfx}",
                              bufs=1)
                if FP8:
                    nc.tensor.matmul(
                        out=sr[:, :F], lhsT=w3p_sb[:, :, 0:1],
                        rhs=h2sa[:, 0:2, :F],
                        start=True, stop=False, perf_mode=PM.DoubleRow,
                    )
                    nc.tensor.matmul(
                        out=sr[:, :F], lhsT=w3s_sb[:, 0:1],
                        rhs=h2sa[:, 2, :F],
                        start=False, stop=True,
                    )
                else:
                    for hb in range(3):
                        nc.tensor.matmul(
                            out=sr[:, :F],
                            lhsT=w3c_sb[:, hb : hb + 1],
                            rhs=h2sa[:, hb, :F],
                            start=(hb == 0),
                            stop=(hb == 2),
                        )
                srow = lsb.tile([1, FMAX], F32, tag="srow_sb",
                                name=f"srow_{bi}{sfx}", bufs=2)
                inv = (1.0 / WSCALE) if FP8 else 1.0
                if int(bi.split("_")[1]) % 2 == 0:
                    nc.vector.tensor_scalar_mul(srow[:, :F], sr[:, :F], inv)
                else:
                    nc.scalar.mul(out=srow[:, :F], in_=sr[:, :F], mul=inv)
                for (k0, G, C, off) in segs:
                    nc.sync.dma_start(
                        out=sblk[k0 : k0 + G, 0:C],
                        in_=srow[0:1, off : off + G * C],
                    )
                if sink is not None:
                    p_, b_ = bi.split("_")
                    row = int(p_) * len(BATCHES) + int(b_)
                    nc.scalar.dma_start(
                        out=sink.ap()[row : row + 1, 0:8],
                        in_=srow[0:1, 0:8],
                    )

            H1DT = FP8DT if FP8 else BF16
            for pass_ in range(mainloop_reps):
                for bi, segs in enumerate(BATCHES):
                    F = _batch_F(segs)
                    h1 = lsb.tile([128, HC, FMAX], H1DT, tag="h1",
                                  name=f"h1_{pass_}_{bi}{sfx}", bufs=3)
                    for c in range(HC):
                        for (k0, G, C, off) in segs:
                            for g in range(G):
                                nc.vector.tensor_scalar(
                                    out=h1[:, c, off + g * C : off + (g + 1) * C],
                                    in0=at_sb[:, c, 0:C],
                                    scalar1=bb_sb[:, c, k0 + g : k0 + g + 1],
                                    scalar2=0.0,
                                    op0=OP.add,
                                    op1=OP.max,
                                )
                    h2sa = lsb.tile([128, 3, FMAX], H1DT, tag="h2sa",
                                    name=f"hs_{pass_}_{bi}{sfx}", bufs=2)
                    for hb in range(3):
                        ph = lps.tile([128, FMAX], F32, tag=f"h2_{hb}",
                                      name=f"ph_{pass_}_{bi}_{hb}{sfx}")
                        if FP8:
                            for kp in range(3):
                                nc.tensor.matmul(
                                    out=ph[:, :F],
                                    lhsT=w2p_sb[:, kp, :,
                                                hb * 128 : (hb + 1) * 128],
                                    rhs=h1[:, 2 * kp : 2 * kp + 2, 0:F],
                                    start=(kp == 0),
                                    stop=(kp == 2),
                                    perf_mode=PM.DoubleRow,
                                )
                        else:
                            for c in range(HC):
                                nc.tensor.matmul(
                                    out=ph[:, :F],
                                    lhsT=w2T_sb[:, c, hb * 128 : (hb + 1) * 128],
                                    rhs=h1[:, c, 0:F],
                                    start=(c == 0),
                                    stop=(c == HC - 1),
                                )
                        nc.scalar.activation(
                            out=h2sa[:, hb, :F], in_=ph[:, :F], func=AF.Relu,
                            bias=b2c_sb[:, hb : hb + 1],
                            scale=(1.0 / WSCALE) if FP8 else 1.0,
                        )
                    if pend is not None:
                        flush_pend()
                    pend = (h2sa, F, segs, f"{pass_}_{bi}")
                    if pass_ == 0:
                        for fn in extras.get(bi, []):
                            fn()
            flush_pend()

            # ---------- tail ----------
            nll_exp(0, 32)
            ln_block()
            pl = ep_tile("pl")
            nc.tensor.matmul(
                out=pl[0:1, 0:1], lhsT=tnll[:, 0:1], rhs=wnll_sb[:],
                start=True, stop=False,
            )
            nc.tensor.matmul(
                out=pl[0:1, 0:1], lhsT=cev[:, 0:1], rhs=wch_sb[:],
                start=False, stop=True,
            )
            lout = cp.tile([1, 1], F32)
            nc.vector.tensor_copy(out=lout[:], in_=pl[0:1, 0:1])
            nc.sync.dma_start(out=T["loss"].ap(), in_=lout[:])


def _build_program(repeat=1, mainloop_reps=1, sink=False):
    nc = bacc.Bacc(
        "TRN2", target_bir_lowering=False, debug=False, enable_asserts=False
    )
    T = _declare_inputs(nc)
    sink_t = None
    if sink:
        sink_t = nc.dram_tensor(
            "sink", [mainloop_reps * len(BATCHES), 8], F32,
            kind="ExternalOutput",
        )
    with tile.TileContext(nc) as tc:
        for rep in range(repeat):
            _emit_core(nc, tc, T, f"_r{rep}" if repeat > 1 else "",
                       mainloop_reps=mainloop_reps, sink=sink_t)
    nc.compile()
    return nc


def _chunk_cols(w):
    """[K, O] -> [128, K//128, O]  (partition-chunked contraction dim)."""
    k, o = w.shape
    return np.ascontiguousarray(w.reshape(k // 128, 128, o).transpose(1, 0, 2))


def _chunk_vec(v, ncol):
    """[C] -> [128, ncol] column-chunks (zero padded)."""
    out = np.zeros((128, ncol), np.float32)
    for c in range(ncol):
        seg = v[c * 128 : (c + 1) * 128]
        out[: len(seg), c] = seg
    return out


def _prep_in_maps(inputs):
    bf = ml_dtypes.bfloat16

    seq = np.asarray(inputs["sequence_output"], np.float32)
    spk = np.asarray(inputs["speaker_emb"], np.float32)
    dummy = np.asarray(inputs["dummy_emb"], np.float32)

    seg = np.asarray(inputs["mentions_seg"]).astype(np.int64)
    mstart = np.asarray(inputs["mention_start"]).astype(np.int64)
    mend = np.asarray(inputs["mention_end"]).astype(np.int64)
    sid = np.asarray(inputs["speaker_ids"]).astype(np.int64)[seg, mstart]
    mention_reps = seq[seg, mstart] + seq[seg, mend] + spk[sid]  # [M, H] f32
    all_reps = np.concatenate([dummy, mention_reps], axis=0)     # [N, H]
    # rT[p, c, m] = all_reps[m, c*128+p]
    rT_np = np.ascontiguousarray(
        all_reps.reshape(N, HC, 128).transpose(2, 1, 0)
    ).astype(bf)                                                 # [128, HC, N]

    W_pair1 = np.asarray(inputs["W_pair1"], np.float32)
    waT = _chunk_cols(np.ascontiguousarray(W_pair1[:, :H].T)).astype(bf)
    wbT = _chunk_cols(np.ascontiguousarray(W_pair1[:, H:].T)).astype(bf)
    w2T = _chunk_cols(
        np.ascontiguousarray(np.asarray(inputs["W_pair2"], np.float32).T)
    )                                                            # [128, 6, 384] f32
    w3c = _chunk_vec(np.asarray(inputs["W_pair3"], np.float32)[0], 3)
    if FP8:
        f8 = mybir.dt.np(FP8DT)
        w2p = np.ascontiguousarray(
            (w2T * WSCALE).reshape(128, 3, 2, H // 2)
        ).astype(f8)                                             # paired k-chunks
        w3p = np.zeros((128, 2, 16), np.float32)
        w3p[:, 0, 0] = w3c[:, 0] * WSCALE
        w3p[:, 1, 0] = w3c[:, 1] * WSCALE
        w3p = w3p.astype(f8)
        w3s = (w3c[:, 2:3] * WSCALE).astype(f8)
    w2T = w2T.astype(bf)
    w3c = w3c.astype(bf)
    b1c = _chunk_vec(np.asarray(inputs["b_pair1"], np.float32), HC)
    b2c = _chunk_vec(np.asarray(inputs["b_pair2"], np.float32), 3)
    wm1T = _chunk_cols(
        np.ascontiguousarray(np.asarray(inputs["W_m1"], np.float32).T)
    ).astype(bf)
    bm1c = _chunk_vec(np.asarray(inputs["b_m1"], np.float32), 3)
    wm2T = _chunk_cols(
        np.ascontiguousarray(np.asarray(inputs["W_m2"], np.float32).T)
    ).astype(bf)
    bm2c = _chunk_vec(np.asarray(inputs["b_m2"], np.float32), 2)
    wm3c = _chunk_vec(np.asarray(inputs["W_m3"], np.float32)[0], 2).astype(bf)
    wc1T = _chunk_cols(
        np.ascontiguousarray(np.asarray(inputs["W_c1"], np.float32).T)
    ).astype(bf)
    bc1c = _chunk_vec(np.asarray(inputs["b_c1"], np.float32), 3)
    wc2T = _chunk_cols(
        np.ascontiguousarray(np.asarray(inputs["W_c2"], np.float32).T)
    ).astype(bf)
    bc2r = np.asarray(inputs["b_c2"], np.float32).reshape(1, 18)

    link_first = np.asarray(inputs["link_first"]).astype(np.int64)
    link_second = np.asarray(inputs["link_second"]).astype(np.int64)
    label = np.asarray(inputs["character_label"]).astype(np.int64)

    mult = np.zeros((N, N), np.float32)
    np.add.at(mult, (link_second, link_first), 1.0)
    has_link = mult.sum(axis=1) > 0
    wnll_full = ((np.arange(N) >= 1) & has_link).astype(np.float32)
    mult[~has_link, 0] = 1.0  # keep log(E) finite; weight is 0 there

    mask_full = np.where(
        np.arange(N)[None, :] >= np.arange(N)[:, None], np.float32(NEG), 0.0
    ).astype(np.float32)

    oneh_full = np.zeros((N, 18), np.float32)
    wch_full = np.zeros(N, np.float32)
    oneh_full[np.arange(1, N), label] = 1.0
    wch_full[1:] = 1.0

    shared = dict(
        rT_in=rT_np,
        wbT=wbT, b1c=b1c, b2c=b2c,
        wm1T=wm1T, bm1c=bm1c, wm2T=wm2T, bm2c=bm2c, wm3c=wm3c,
        wc1T=wc1T, bc1c=bc1c, wc2T=wc2T, bc2r=bc2r,
    )
    if FP8:
        shared.update(w2p=w2p, w3p=w3p, w3s=w3s)
    else:
        shared.update(w2T=w2T, w3c=w3c)
    for co in range(HC):
        shared[f"waT{co}"] = np.ascontiguousarray(
            waT[:, :, co * 128 : (co + 1) * 128]
        )
    in_maps = []
    for d in range(NC_):
        rows = np.arange(R) * NC_ + d      # modulo sharding: row 8k+d
        m = dict(shared)
        m["rTl_in"] = np.ascontiguousarray(rT_np[:, :, rows])
        m["maskb"] = np.ascontiguousarray(mask_full[rows])
        if d == 0:
            # Global row 0 is fully masked; without row-max subtraction its
            # Z would be exactly 0 (ln -> -inf). Unmask its [0,0] entry:
            # mult[0,0]=1 and wnll[0]=0 make the row contribute exactly 0.
            m["maskb"][0, 0] = 0.0
        m["multb"] = np.ascontiguousarray(mult[rows])
        m["wnll"] = np.ascontiguousarray(wnll_full[rows]).reshape(R, 1)
        m["oneh"] = np.ascontiguousarray(oneh_full[rows])
        m["wch"] = np.ascontiguousarray(wch_full[rows]).reshape(R, 1)
        in_maps.append(m)
    return in_maps


def kernel(**inputs):
    global LAST_RESULT
    in_maps = _prep_in_maps(inputs)

    if "nc" not in _CACHE:
        _CACHE["nc"] = _build_program()
    nc = _CACHE["nc"]

    res = run_bass_kernel_spmd(nc, in_maps, core_ids=list(range(NC_)))
    LAST_RESULT = res
    total = np.float32(0.0)
    for d in range(NC_):
        total += np.float32(res.results[d]["loss"][0, 0])
    return np.asarray(total, dtype=np.float32)


if __name__ == "__main__":
    import reference

    inputs = {k: np.asarray(v) for k, v in reference.setup_inputs().items()}
    out = kernel(**inputs)
    print("kernel out:", out)
